# revision 1
# baseline (speedup 1.0000x reference)
# MiniGPT (L=6, D=1024, Dff=4096, S=2048, V=32000, 16 heads) on 8 trn2 NeuronCores.
#
# Sharding: sequence-sharded fp32 residual stream (256 rows/core); attention
# head-sharded (2 heads/core, Megatron column QKV / row Wo); FFN
# sequence-sharded with full streamed weights (no collective); per layer one
# AllGather (of layernormed activations, bf16) + one ReduceScatter (of Wo
# partials, bf16); LM head vocab-sharded (4000 cols/core) after a final
# AllGather; logits concatenated on host.
import numpy as np
import ml_dtypes

V, D, DFF, L, S = 32000, 1024, 4096, 6, 2048
NCORES = 8
NH, DK = 16, 64
SSH = S // NCORES          # 256 sequence rows per core
HPC = NH // NCORES         # 2 heads per core
VSH = V // NCORES          # 4000 vocab cols per core
P = 128
EPS = 1e-5
BF16 = ml_dtypes.bfloat16

_cache = {}


def build(layers=L):
    import concourse.mybir as mybir
    import concourse.tile as tile
    from concourse import bacc
    import concourse.bass as bass
    from concourse.masks import make_identity

    fp32 = mybir.dt.float32
    bf16 = mybir.dt.bfloat16
    i32 = mybir.dt.int32

    nc = bacc.Bacc("TRN2", target_bir_lowering=False, debug=False,
                   num_devices=NCORES)

    # ---------------- I/O ----------------
    tok_in = nc.dram_tensor("tokens_sh", [SSH], i32, kind="ExternalInput")
    emb_in = nc.dram_tensor("emb_g", [V, D], bf16, kind="ExternalInput")
    pos_in = nc.dram_tensor("pos_sh", [SSH, D], fp32, kind="ExternalInput")
    wq_in = nc.dram_tensor("wq", [layers, D, P], bf16, kind="ExternalInput")
    wk_in = nc.dram_tensor("wk", [layers, D, P], bf16, kind="ExternalInput")
    wv_in = nc.dram_tensor("wv", [layers, D, P], bf16, kind="ExternalInput")
    wo_in = nc.dram_tensor("wo", [layers, P, D], bf16, kind="ExternalInput")
    bq_in = nc.dram_tensor("bq", [layers, P], fp32, kind="ExternalInput")
    bk_in = nc.dram_tensor("bk", [layers, P], fp32, kind="ExternalInput")
    bv_in = nc.dram_tensor("bv", [layers, P], fp32, kind="ExternalInput")
    bo_in = nc.dram_tensor("bo", [layers, D], fp32, kind="ExternalInput")
    w1_in = nc.dram_tensor("w1", [layers, D, DFF], bf16, kind="ExternalInput")
    b1_in = nc.dram_tensor("b1", [layers, DFF], fp32, kind="ExternalInput")
    w2_in = nc.dram_tensor("w2", [layers, DFF, D], bf16, kind="ExternalInput")
    b2_in = nc.dram_tensor("b2", [layers, D], fp32, kind="ExternalInput")
    lnfg_in = nc.dram_tensor("lnf_g", [D], fp32, kind="ExternalInput")
    lnfb_in = nc.dram_tensor("lnf_b", [D], fp32, kind="ExternalInput")
    embt_in = nc.dram_tensor("embt", [D, VSH], bf16, kind="ExternalInput")
    mask_in = nc.dram_tensor("mask", [P, 896], bf16, kind="ExternalInput")
    chain_in = nc.dram_tensor("chain", [1, 64], fp32, kind="ExternalInput")
    out_t = nc.dram_tensor("logits_sh", [S, VSH], fp32, kind="ExternalOutput")
    chain_out = nc.dram_tensor("chain_out", [1, 64], fp32,
                               kind="ExternalOutput")

    DSUB = D // P              # 8
    NST = SSH // P             # 2 sequence tiles per core shard
    NQT = S // P               # 16 q tiles
    NQB = S // 512             # 4 q blocks of 512
    NKT = S // P               # 16 k tiles
    NT1 = DFF // P             # 32 dff tiles

    def bcast_ap(handle, off, n):
        # replicate a [n]-f32 DRAM row across all partitions
        return bass.AP(tensor=handle, offset=off, ap=[[0, P], [1, n]])

    with tile.TileContext(nc) as tc:
        with (
            tc.tile_pool(name="const", bufs=1) as const,
            tc.tile_pool(name="persist", bufs=1) as persist,
            tc.tile_pool(name="big", bufs=1) as big,
            tc.tile_pool(name="hbuf", bufs=2) as hbuf,
            tc.tile_pool(name="wpool", bufs=2) as wpool,
            tc.tile_pool(name="stream", bufs=2) as stream,
            tc.tile_pool(name="streamw", bufs=3) as streamw,
            tc.tile_pool(name="small", bufs=2) as small,
            tc.tile_pool(name="ptile", bufs=4) as ptile,
            tc.tile_pool(name="outc", bufs=2) as outc,
            tc.tile_pool(name="psA", bufs=2, space="PSUM") as psA,
            tc.tile_pool(name="psS", bufs=2, space="PSUM") as psS,
            tc.tile_pool(name="psC", bufs=2, space="PSUM") as psC,
            tc.tile_pool(name="dram", bufs=2, space="DRAM") as dram,
            tc.tile_pool(name="dsmall", bufs=4, space="DRAM") as dsmall,
        ):
            # ---------------- constants ----------------
            eps_t = const.tile([P, 1], fp32, tag="eps")
            nc.vector.memset(eps_t, EPS)
            ident = const.tile([P, P], bf16, tag="ident")
            make_identity(nc, ident[:])
            mask_sb = const.tile([P, 896], bf16, tag="mask")
            nc.sync.dma_start(out=mask_sb[:], in_=mask_in[:])

            x_sb = persist.tile([P, NST, D], fp32, tag="x")

            # ---------------- embedding ----------------
            toks = small.tile([P, NST], i32, tag="toks")
            nc.sync.dma_start(out=toks[:],
                              in_=tok_in.ap().rearrange("(st p) -> p st", p=P))
            pos_sb = persist.tile([P, NST, D], fp32, tag="pos")
            nc.sync.dma_start(out=pos_sb[:],
                              in_=pos_in.ap().rearrange("(st p) d -> p st d", p=P))
            for st in range(NST):
                ga_b = small.tile([P, D], bf16, tag="gab")
                nc.gpsimd.indirect_dma_start(
                    out=ga_b[:], out_offset=None, in_=emb_in[:],
                    in_offset=bass.IndirectOffsetOnAxis(ap=toks[:, st:st + 1],
                                                        axis=0))
                ga_f = small.tile([P, D], fp32, tag="gaf")
                nc.vector.tensor_copy(out=ga_f[:], in_=ga_b[:])
                nc.vector.tensor_add(out=x_sb[:, st], in0=ga_f[:],
                                     in1=pos_sb[:, st])

            # layernorm (no affine): writes bf16 h tile [P, D]
            def layer_norm(dst, src):
                stats = small.tile([P, 2, nc.vector.BN_STATS_DIM], fp32,
                                   tag="lnstats")
                mv = small.tile([P, nc.vector.BN_AGGR_DIM], fp32, tag="lnmv")
                srcg = src.rearrange("p (g d) -> p g d", g=2)
                for g in range(2):
                    nc.vector.bn_stats(out=stats[:, g], in_=srcg[:, g])
                nc.vector.bn_aggr(out=mv[:], in_=stats[:])
                mean, var = mv[:, 0:1], mv[:, 1:2]
                nc.scalar.activation(out=var, in_=var,
                                     func=mybir.ActivationFunctionType.Sqrt,
                                     bias=eps_t[:], scale=1.0)
                nc.vector.reciprocal(out=var, in_=var)
                nc.vector.tensor_scalar(out=dst, in0=src, scalar1=mean,
                                        scalar2=var,
                                        op0=mybir.AluOpType.subtract,
                                        op1=mybir.AluOpType.mult)

            # transpose own-shard [P, NST, D] bf16 -> hTown [P, DSUB, NST, P]
            def transpose_shard(h_t):
                hTown = hbuf.tile([P, DSUB, NST, P], bf16, tag="hTown")
                for st in range(NST):
                    for ds in range(DSUB):
                        ptr = psA.tile([P, P], bf16, tag="mm")
                        nc.tensor.transpose(ptr[:],
                                            h_t[:, st, ds * P:(ds + 1) * P],
                                            ident[:])
                        nc.any.tensor_copy(out=hTown[:, ds, st, :], in_=ptr[:])
                return hTown

            # AG own transposed shard -> full hT [P, DSUB, S]
            def allgather_hT(hTown, tagsuffix=""):
                ag_in = dram.tile([D, SSH], bf16, tag="ag_in")
                ag_out = dram.tile([NCORES * D, SSH], bf16,
                                   addr_space="Shared", tag="ag_out")
                nc.sync.dma_start(
                    out=ag_in.rearrange("(ds p) (st j) -> p ds st j",
                                             p=P, st=NST),
                    in_=hTown[:])
                nc.gpsimd.collective_compute(
                    "AllGather", mybir.AluOpType.bypass,
                    replica_groups=[list(range(NCORES))],
                    ins=[ag_in[:]], outs=[ag_out[:]])
                hT = big.tile([P, DSUB, S], bf16, tag="hT")
                for r in range(NCORES):
                    nc.sync.dma_start(
                        out=hT[:, :, r * SSH:(r + 1) * SSH],
                        in_=ag_out[r * D:(r + 1) * D, :]
                        .rearrange("(ds p) s -> p ds s", p=P))
                return hT

            # ---------------- transformer layers ----------------
            for l in range(layers):
                # --- ln1 + transpose + allgather
                h_t = hbuf.tile([P, NST, D], bf16, tag="h")
                for st in range(NST):
                    layer_norm(h_t[:, st], x_sb[:, st])
                hTown = transpose_shard(h_t)
                hT = allgather_hT(hTown)

                # --- weights for attention
                wq_sb = wpool.tile([P, DSUB, P], bf16, tag="wq")
                nc.sync.dma_start(
                    out=wq_sb[:],
                    in_=wq_in[l].rearrange("(ds p) m -> p ds m", p=P))
                wk_sb = wpool.tile([P, DSUB, P], bf16, tag="wk")
                nc.sync.dma_start(
                    out=wk_sb[:],
                    in_=wk_in[l].rearrange("(ds p) m -> p ds m", p=P))
                wv_sb = wpool.tile([P, DSUB, P], bf16, tag="wv")
                nc.sync.dma_start(
                    out=wv_sb[:],
                    in_=wv_in[l].rearrange("(ds p) m -> p ds m", p=P))
                bq_t = small.tile([P, 1], fp32, tag="bq")
                nc.sync.dma_start(out=bq_t[:],
                                  in_=bq_in[l].rearrange("(p o) -> p o", o=1))
                bk_t = small.tile([P, 1], fp32, tag="bk")
                nc.sync.dma_start(out=bk_t[:],
                                  in_=bk_in[l].rearrange("(p o) -> p o", o=1))
                bv_rep = small.tile([P, P], fp32, tag="bvrep")
                nc.sync.dma_start(out=bv_rep[:],
                                  in_=bcast_ap(bv_in, l * P, P))

                # --- QT / KT : [P(2 heads x dk), S] bf16
                qt_sb = big.tile([P, S], bf16, tag="qt")
                kt_sb = big.tile([P, S], bf16, tag="kt")
                for dst, w_sb, b_t in ((qt_sb, wq_sb, bq_t),
                                       (kt_sb, wk_sb, bk_t)):
                    for sb in range(NQB):
                        pq = psA.tile([P, 512], fp32, tag="mm")
                        for ds in range(DSUB):
                            nc.tensor.matmul(
                                pq[:], lhsT=w_sb[:, ds, :],
                                rhs=hT[:, ds, sb * 512:(sb + 1) * 512],
                                start=(ds == 0), stop=(ds == DSUB - 1))
                        nc.vector.tensor_scalar(
                            out=dst[:, sb * 512:(sb + 1) * 512], in0=pq[:],
                            scalar1=b_t[:, 0:1], scalar2=None,
                            op0=mybir.AluOpType.add)

                # --- V (natural layout + ones cols): [P, NKT, 132]
                v_sb = big.tile([P, NKT, 132], bf16, tag="v")
                nc.vector.memset(v_sb[:, :, 64:65], 1.0)
                nc.vector.memset(v_sb[:, :, 130:131], 1.0)
                for kt in range(NKT):
                    pv = psA.tile([P, P], fp32, tag="mm")
                    for ds in range(DSUB):
                        nc.tensor.matmul(pv[:],
                                         lhsT=hT[:, ds, kt * P:(kt + 1) * P],
                                         rhs=wv_sb[:, ds, :],
                                         start=(ds == 0),
                                         stop=(ds == DSUB - 1))
                    nc.vector.tensor_tensor(out=v_sb[:, kt, 0:64],
                                            in0=pv[:, 0:64],
                                            in1=bv_rep[:, 0:64],
                                            op=mybir.AluOpType.add)
                    nc.vector.tensor_tensor(out=v_sb[:, kt, 66:130],
                                            in0=pv[:, 64:128],
                                            in1=bv_rep[:, 64:128],
                                            op=mybir.AluOpType.add)

                # --- attention: scores^T -> exp/mask -> ctx^T (augmented)
                # Both heads share one [P, 2, 512] scores psum: the two score
                # matmuls use disjoint PE row groups (lhsT base partitions 0 /
                # 64) so they run concurrently, and exp+mask become single ops
                # over both heads.
                ctxT = big.tile([P, S], bf16, tag="ctxT")
                for qb in range(NQB):
                    nkt = 4 * qb + 4          # k tiles needed for this q block
                    pctxs = []
                    for h in range(HPC):
                        pc = psC.tile([65, 512], fp32, tag="ctx",
                                      name=f"pctx{h}")
                        pctxs.append(pc)
                    for kt in range(nkt):
                        ps = psS.tile([P, 2, 512], fp32, tag="s")
                        for h in range(HPC):
                            nc.tensor.matmul(
                                ps[:, h],
                                lhsT=kt_sb[64 * h:64 * h + 64,
                                           kt * P:(kt + 1) * P],
                                rhs=qt_sb[64 * h:64 * h + 64,
                                          qb * 512:(qb + 1) * 512],
                                start=True, stop=True)
                        p_t = ptile.tile([P, 2, 512], bf16, tag="p")
                        nc.scalar.activation(
                            out=p_t[:], in_=ps[:],
                            func=mybir.ActivationFunctionType.Exp)
                        d = kt * P - qb * 512
                        if d >= 0:
                            msl = mask_sb[:, 384 - d:896 - d]
                            m2 = bass.AP(tensor=msl.tensor, offset=msl.offset,
                                         ap=[list(msl.ap[0]), [0, 2],
                                             list(msl.ap[1])])
                            nc.vector.tensor_tensor(
                                out=p_t[:], in0=p_t[:], in1=m2,
                                op=mybir.AluOpType.mult)
                        for h in range(HPC):
                            nc.tensor.matmul(pctxs[h][:],
                                             lhsT=v_sb[:, kt,
                                                       66 * h:66 * h + 65],
                                             rhs=p_t[:, h],
                                             start=(kt == 0),
                                             stop=(kt == nkt - 1))
                    # normalize by denominator (row 64) via DRAM bcast
                    for h in range(HPC):
                        pctx = pctxs[h]
                        recip = small.tile([1, 512], fp32, tag="recip")
                        nc.vector.reciprocal(out=recip[:], in_=pctx[64:65, :])
                        rd = dsmall.tile([1, 512], fp32, tag="rd")
                        nc.sync.dma_start(out=rd[:], in_=recip[:])
                        rrep = small.tile([64, 512], fp32, tag="rrep")
                        nc.sync.dma_start(
                            out=rrep[:],
                            in_=bass.AP(tensor=rd.tensor, offset=rd.offset,
                                        ap=[[0, 64], [1, 512]]))
                        nc.vector.tensor_mul(
                            out=ctxT[64 * h:64 * h + 64,
                                     qb * 512:(qb + 1) * 512],
                            in0=pctx[0:64, :], in1=rrep[:])

                # --- Wo partials + ReduceScatter
                wo_sb = wpool.tile([P, D], bf16, tag="wo")
                nc.sync.dma_start(out=wo_sb[:], in_=wo_in[l])
                rs_in = dram.tile([S, D], bf16, tag="rs_in")
                rs_out = dram.tile([SSH, D], bf16, tag="rs_out")
                for qt in range(NQT):
                    for db in range(2):
                        py = psA.tile([P, 512], fp32, tag="mm")
                        nc.tensor.matmul(py[:],
                                         lhsT=ctxT[:, qt * P:(qt + 1) * P],
                                         rhs=wo_sb[:, db * 512:(db + 1) * 512],
                                         start=True, stop=True)
                        yp = outc.tile([P, 512], bf16, tag="yp")
                        nc.any.tensor_copy(out=yp[:], in_=py[:])
                        nc.sync.dma_start(
                            out=rs_in[qt * P:(qt + 1) * P,
                                      db * 512:(db + 1) * 512],
                            in_=yp[:])
                nc.gpsimd.collective_compute(
                    "ReduceScatter", mybir.AluOpType.add,
                    replica_groups=[list(range(NCORES))],
                    ins=[rs_in[:]], outs=[rs_out[:]])
                ysh = small.tile([P, NST, D], bf16, tag="ysh")
                nc.sync.dma_start(
                    out=ysh[:],
                    in_=rs_out.rearrange("(st p) d -> p st d", p=P))
                bo_rep = small.tile([P, D], fp32, tag="borep")
                nc.sync.dma_start(out=bo_rep[:], in_=bcast_ap(bo_in, l * D, D))
                for st in range(NST):
                    nc.vector.tensor_add(out=x_sb[:, st], in0=x_sb[:, st],
                                         in1=ysh[:, st])
                    nc.vector.tensor_add(out=x_sb[:, st], in0=x_sb[:, st],
                                         in1=bo_rep[:])

                # --- FFN (sequence-sharded, weights streamed)
                h2 = hbuf.tile([P, NST, D], bf16, tag="h")
                for st in range(NST):
                    layer_norm(h2[:, st], x_sb[:, st])
                h2Town = transpose_shard(h2)
                h2T = h2Town.rearrange("p ds st j -> p ds (st j)")
                b1_t = small.tile([P, NT1], fp32, tag="b1")
                nc.sync.dma_start(out=b1_t[:],
                                  in_=b1_in[l].rearrange("(t p) -> p t", p=P))
                yps = []
                for i in range(4):
                    pool_, tg = (psC, "ctx") if i < 2 else (psS, "s")
                    ypsi = pool_.tile([P, 512], fp32, tag=tg, name=f"yps{i}")
                    yps.append(ypsi)
                for t in range(NT1):
                    w1t = streamw.tile([P, DSUB, P], bf16, tag="w1t")
                    nc.sync.dma_start(
                        out=w1t[:],
                        in_=w1_in[l, :, t * P:(t + 1) * P]
                        .rearrange("(ds p) m -> p ds m", p=P))
                    w2t = streamw.tile([P, D], bf16, tag="w2t")
                    nc.sync.dma_start(out=w2t[:],
                                      in_=w2_in[l, t * P:(t + 1) * P, :])
                    pa = psA.tile([P, SSH], fp32, tag="mm")
                    for ds in range(DSUB):
                        nc.tensor.matmul(pa[:], lhsT=w1t[:, ds, :],
                                         rhs=h2T[:, ds, :],
                                         start=(ds == 0),
                                         stop=(ds == DSUB - 1))
                    gt = outc.tile([P, SSH], bf16, tag="gt")
                    nc.scalar.activation(out=gt[:], in_=pa[:],
                                         func=mybir.ActivationFunctionType.Gelu,
                                         bias=b1_t[:, t:t + 1], scale=1.0)
                    for st in range(NST):
                        for db in range(2):
                            nc.tensor.matmul(
                                yps[st * 2 + db][:],
                                lhsT=gt[:, st * P:(st + 1) * P],
                                rhs=w2t[:, db * 512:(db + 1) * 512],
                                start=(t == 0), stop=(t == NT1 - 1))
                b2_rep = small.tile([P, D], fp32, tag="b2rep")
                nc.sync.dma_start(out=b2_rep[:], in_=bcast_ap(b2_in, l * D, D))
                for st in range(NST):
                    for db in range(2):
                        nc.vector.tensor_add(
                            out=x_sb[:, st, db * 512:(db + 1) * 512],
                            in0=x_sb[:, st, db * 512:(db + 1) * 512],
                            in1=yps[st * 2 + db][:])
                    nc.vector.tensor_add(out=x_sb[:, st], in0=x_sb[:, st],
                                         in1=b2_rep[:])

            # ---------------- final LN + LM head ----------------
            lnfg_rep = const.tile([P, D], fp32, tag="lnfg")
            nc.sync.dma_start(out=lnfg_rep[:], in_=bcast_ap(lnfg_in, 0, D))
            lnfb_rep = const.tile([P, D], fp32, tag="lnfb")
            nc.sync.dma_start(out=lnfb_rep[:], in_=bcast_ap(lnfb_in, 0, D))
            xf = hbuf.tile([P, NST, D], bf16, tag="h")
            for st in range(NST):
                xc = small.tile([P, D], fp32, tag="xc")
                layer_norm(xc[:], x_sb[:, st])
                nc.vector.tensor_mul(out=xc[:], in0=xc[:], in1=lnfg_rep[:])
                nc.vector.tensor_add(out=xf[:, st], in0=xc[:], in1=lnfb_rep[:])
            xfTown = transpose_shard(xf)
            xfT = allgather_hT(xfTown)

            VB = 8
            VBS = VSH // VB           # 500
            for vb in range(VB):
                et = stream.tile([P, DSUB, VBS], bf16, tag="et")
                nc.sync.dma_start(
                    out=et[:],
                    in_=embt_in[:, vb * VBS:(vb + 1) * VBS]
                    .rearrange("(ds p) v -> p ds v", p=P))
                for qt in range(NQT):
                    pl = psA.tile([P, VBS], fp32, tag="mm")
                    for ds in range(DSUB):
                        nc.tensor.matmul(pl[:],
                                         lhsT=xfT[:, ds, qt * P:(qt + 1) * P],
                                         rhs=et[:, ds, :],
                                         start=(ds == 0),
                                         stop=(ds == DSUB - 1))
                    lo = outc.tile([P, VBS], fp32, tag="lo")
                    nc.any.tensor_copy(out=lo[:], in_=pl[:])
                    nc.sync.dma_start(
                        out=out_t[qt * P:(qt + 1) * P,
                                  vb * VBS:(vb + 1) * VBS],
                        in_=lo[:])
                    if vb == VB - 1 and qt == NQT - 1:
                        # timing chain: depends on the final logits tile so
                        # chain_out completion implies full kernel completion
                        ch = small.tile([1, 64], fp32, tag="chain")
                        nc.sync.dma_start(out=ch[:], in_=chain_in[:])
                        nc.vector.tensor_add(out=ch[:], in0=ch[:],
                                             in1=lo[0:1, 0:64])
                        nc.sync.dma_start(out=chain_out[:], in_=ch[:])

    nc.compile()
    return nc


def prep_inputs(inputs, layers=L):
    """Host-side shard/cast prep. Returns per-core in_maps."""
    tokens = np.asarray(inputs["tokens"]).reshape(-1).astype(np.int32)
    tok_emb = np.asarray(inputs["tok_emb"], dtype=np.float32)
    pos_emb = np.asarray(inputs["pos_emb"], dtype=np.float32)
    Wq = np.asarray(inputs["Wq"], dtype=np.float32)
    Wk = np.asarray(inputs["Wk"], dtype=np.float32)
    Wv = np.asarray(inputs["Wv"], dtype=np.float32)
    Wo = np.asarray(inputs["Wo"], dtype=np.float32)
    bq = np.asarray(inputs["bq"], dtype=np.float32)
    bk = np.asarray(inputs["bk"], dtype=np.float32)
    bv = np.asarray(inputs["bv"], dtype=np.float32)
    bo = np.asarray(inputs["bo"], dtype=np.float32)
    ln1_g = np.asarray(inputs["ln1_g"], dtype=np.float32)
    ln1_b = np.asarray(inputs["ln1_b"], dtype=np.float32)
    ln2_g = np.asarray(inputs["ln2_g"], dtype=np.float32)
    ln2_b = np.asarray(inputs["ln2_b"], dtype=np.float32)
    W1 = np.asarray(inputs["W1"], dtype=np.float32)
    b1 = np.asarray(inputs["b1"], dtype=np.float32)
    W2 = np.asarray(inputs["W2"], dtype=np.float32)
    b2 = np.asarray(inputs["b2"], dtype=np.float32)
    lnf_g = np.asarray(inputs["lnf_g"], dtype=np.float32)
    lnf_b = np.asarray(inputs["lnf_b"], dtype=np.float32)

    scale = 1.0 / np.sqrt(DK)
    emb_bf = tok_emb.astype(BF16)
    embt = tok_emb.T.copy()  # [D, V] f32
    mask = (np.arange(P)[:, None] <= np.arange(896)[None, :] - 384)
    mask = mask.astype(BF16)

    in_maps = []
    for c in range(NCORES):
        hs = slice(P * c, P * (c + 1))         # 2-head col slice of D
        # fold ln gains into the consuming matmuls; ln biases fold into the
        # matmul bias terms (b + ln_b @ W). Fold the 1/sqrt(dk) score scale
        # into Wq/bq (exact: 0.125 is a power of two).
        wq_c = np.empty((layers, D, P), dtype=BF16)
        wk_c = np.empty((layers, D, P), dtype=BF16)
        wv_c = np.empty((layers, D, P), dtype=BF16)
        wo_c = np.empty((layers, P, D), dtype=BF16)
        w1_c = np.empty((layers, D, DFF), dtype=BF16)
        w2_c = np.empty((layers, DFF, D), dtype=BF16)
        bq_c = np.empty((layers, P), dtype=np.float32)
        bk_c = np.empty((layers, P), dtype=np.float32)
        bv_c = np.empty((layers, P), dtype=np.float32)
        b1_c = np.empty((layers, DFF), dtype=np.float32)
        for l in range(layers):
            g1 = ln1_g[l][:, None]
            wq_c[l] = (g1 * Wq[l][:, hs] * scale).astype(BF16)
            wk_c[l] = (g1 * Wk[l][:, hs]).astype(BF16)
            wv_c[l] = (g1 * Wv[l][:, hs]).astype(BF16)
            bq_c[l] = (bq[l][hs] + ln1_b[l] @ Wq[l][:, hs]) * scale
            bk_c[l] = bk[l][hs] + ln1_b[l] @ Wk[l][:, hs]
            bv_c[l] = bv[l][hs] + ln1_b[l] @ Wv[l][:, hs]
            wo_c[l] = Wo[l][hs, :].astype(BF16)
            w1_c[l] = (ln2_g[l][:, None] * W1[l]).astype(BF16)
            b1_c[l] = b1[l] + ln2_b[l] @ W1[l]
            w2_c[l] = W2[l].astype(BF16)
        in_maps.append({
            "tokens_sh": tokens[SSH * c:SSH * (c + 1)].copy(),
            "emb_g": emb_bf,
            "pos_sh": pos_emb[SSH * c:SSH * (c + 1)].copy(),
            "wq": wq_c, "wk": wk_c, "wv": wv_c, "wo": wo_c,
            "bq": bq_c, "bk": bk_c, "bv": bv_c,
            "bo": bo[:layers].copy(),
            "w1": w1_c, "b1": b1_c, "w2": w2_c,
            "b2": b2[:layers].copy(),
            "lnf_g": lnf_g, "lnf_b": lnf_b,
            "embt": embt[:, VSH * c:VSH * (c + 1)].astype(BF16),
            "mask": mask,
            "chain": np.zeros((1, 64), dtype=np.float32),
        })
    return in_maps


def run(inputs, layers=L, **run_kwargs):
    from concourse.bass_utils import run_bass_kernel_spmd
    key = layers
    if key not in _cache:
        _cache[key] = build(layers)
    nc = _cache[key]
    in_maps = prep_inputs(inputs, layers)
    res = run_bass_kernel_spmd(nc, in_maps, core_ids=list(range(NCORES)),
                               **run_kwargs)
    logits = np.concatenate([res.results[c]["logits_sh"]
                             for c in range(NCORES)], axis=1)
    return logits[None], res


def kernel(**inputs):
    logits, _ = run(inputs)
    return logits

